# revision 10
# baseline (speedup 1.0000x reference)
"""Banded local-linear layer (nn_LocalLinearLayer) on 8 trn2 NeuronCores.

out[b, o, c] = sum_p W[o, p] * xpad[b, c, p] + bias[o],  band p in [o, o+25)
xpad = concat(x[:12], x, x[-12:]) along L (first/last 12 rows duplicated).

Strategy (v9, tensor-parallel over L, single matmul per tile, warmed PE):
  - Each core owns 512 output rows (L/8); free dim = all B*C = 2048 cols.
    L-sharded weights keep replicated-weight HBM traffic tiny.
  - Output tiled in M=104-row tiles (5 per core: 4x104 + 1x96); tile t
    contracts over xpad rows [104t, 104t+128) -> ONE K=128 matmul per
    512-col PSUM chunk. Host pre-shuffles x into partition-aligned tile
    layout (23% duplicate HBM bytes, traded for halved PE stream count —
    measured: K-split pairs run at ~318ns/mm vs 216ns single).
  - PE HAM warm-up: ~4.3us of dummy matmuls on a zeroed scratch tile while
    the first x DMAs are in flight, so real matmuls run at 2.4GHz.
  - One 4-bank PSUM tile per output tile (4 matmuls into its bank slices),
    drained by TWO large copies: ScalarE activation on cols [0:768] and
    VectorE tensor_scalar on [768:2048] (both add bias, cast f32->f16).
    Large ops amortize the ~270ns per-op engine overhead.
  - fp16 operands + fp16 output (fp32 PSUM accum, fp32 bias).
  - x + w1 on the Sync HWDGE ring; bias + per-tile output stores on the
    Scalar ring so stores overlap the input stream.
"""

import sys

for _p in ("/opt/trn_rl_repo",):
    if _p not in sys.path:
        sys.path.insert(0, _p)

import numpy as np

import concourse.bass as bass
import concourse.tile as tile
from concourse import bacc, mybir
from concourse.bass_utils import run_bass_kernel_spmd

L = 4096
WIN = 25
PAD = (WIN - 1) // 2  # 12
PADDED = L + 2 * PAD  # 4120
B = 32
C = 64
NCORES = 8
P = 128
RPC = L // NCORES  # 512 output rows per core
M = 104  # output rows per tile (K = M + WIN - 1 = 128)
NT = (RPC + M - 1) // M  # 5 tiles per core
M_LAST = RPC - (NT - 1) * M  # 96
K_LAST = M_LAST + WIN - 1  # 120
NF = B * C  # 2048 free columns
NCH = 4
CHUNK = NF // NCH  # 512 (one PSUM bank of fp32)
ASPLIT = 768  # ScalarE copies cols [0:768], VectorE [768:2048]
NWARM = 10  # dummy matmuls to flip the PE HAM clock gate to 2.4GHz

F32 = mybir.dt.float32
F16 = mybir.dt.float16


def _host_weights(W: np.ndarray, b: np.ndarray):
    """w1[c][k, t, m] = Wm[base+m, base+k], bias[c][m, t] = b[base+m],
    base = 512c + 104t."""
    o = np.arange(L)[:, None]
    p = np.arange(PADDED)[None, :]
    Wm = np.where((p >= o) & (p < o + WIN), W, 0.0).astype(np.float32)
    w1 = np.zeros((NCORES, P, NT, M), np.float16)
    bias = np.zeros((NCORES, M, NT), np.float32)
    for c in range(NCORES):
        for t in range(NT):
            base = RPC * c + M * t
            mt = M if t < NT - 1 else M_LAST
            kt = P if t < NT - 1 else K_LAST
            w1[c, :kt, t, :mt] = Wm[base : base + mt, base : base + kt].T
            bias[c, :mt, t] = b[base : base + mt]
    return w1, bias


def _host_x(x: np.ndarray):
    """x [B, L, C] f32 -> per-core [P, NT, NF] f16 tile layout,
    xc[c][k, t, f] = xpad[b, 512c + 104t + k, ch]  (f = 64b + ch)."""
    xp = np.concatenate([x[:, :PAD], x, x[:, -PAD:]], axis=1).astype(np.float16)
    xcs = []
    for c in range(NCORES):
        xc = np.zeros((P, NT, NF), np.float16)
        for t in range(NT):
            base = RPC * c + M * t
            kt = P if t < NT - 1 else K_LAST
            xc[:kt, t] = xp[:, base : base + kt].transpose(1, 0, 2).reshape(kt, NF)
        xcs.append(xc)
    return xcs


def _build_nc():
    nc = bacc.Bacc("TRN2", target_bir_lowering=False, debug=False, num_devices=NCORES)
    xm_d = nc.dram_tensor("xm", [P, NT, NF], F16, kind="ExternalInput").ap()
    w1_d = nc.dram_tensor("w1", [P, NT, M], F16, kind="ExternalInput").ap()
    bias_d = nc.dram_tensor("bias", [M, NT], F32, kind="ExternalInput").ap()
    out_d = nc.dram_tensor("out", [M, NT, NF], F16, kind="ExternalOutput").ap()

    with tile.TileContext(nc) as tc:
        with (
            tc.tile_pool(name="main", bufs=1) as pool,
            tc.tile_pool(name="ps", bufs=2, space=bass.MemorySpace.PSUM) as pspool,
        ):
            w1_s = pool.tile([P, NT, M], F16)
            bias_s = pool.tile([M, NT], F32)
            scr = pool.tile([P, CHUNK], F16)
            xs = [pool.tile([P, NF], F16, name=f"x{t}") for t in range(NT)]
            obs = [pool.tile([M, NF], F16, name=f"o{t}") for t in range(NT)]

            nc.sync.dma_start(xs[0][:], xm_d[:, 0, :])
            nc.sync.dma_start(w1_s[:], w1_d)
            for t in range(1, NT):
                nc.sync.dma_start(xs[t][:], xm_d[:, t, :])
            nc.scalar.dma_start(bias_s[:], bias_d)

            # PE HAM warm-up while the x stream is in flight
            nc.vector.memset(scr[:], 0.0)
            for _ in range(NWARM):
                wps = pspool.tile([M, NF], F32, name="ps")
                nc.tensor.matmul(wps[:, :CHUNK], scr[:, :M], scr[:], start=True, stop=True)

            for t in range(NT):
                mt = M if t < NT - 1 else M_LAST
                kt = P if t < NT - 1 else K_LAST
                ps = pspool.tile([M, NF], F32, name="ps")
                for j in range(NCH):
                    sl = slice(j * CHUNK, (j + 1) * CHUNK)
                    nc.tensor.matmul(
                        ps[:mt, sl],
                        w1_s[:kt, t, :mt],
                        xs[t][:kt, sl],
                        start=True,
                        stop=True,
                    )
                nc.scalar.activation(
                    obs[t][:mt, :ASPLIT],
                    ps[:mt, :ASPLIT],
                    mybir.ActivationFunctionType.Identity,
                    bias=bias_s[:mt, t : t + 1],
                )
                nc.vector.tensor_scalar_add(
                    obs[t][:mt, ASPLIT:], ps[:mt, ASPLIT:], bias_s[:mt, t : t + 1]
                )
                nc.scalar.dma_start(out_d[:mt, t, :], obs[t][:mt, :])

    nc.compile()
    return nc


_NC = None


def _get_nc():
    global _NC
    if _NC is None:
        _NC = _build_nc()
    return _NC


def _make_in_maps(x, W, b):
    w1, bias = _host_weights(
        np.asarray(W, dtype=np.float32), np.asarray(b, dtype=np.float32)
    )
    xcs = _host_x(np.asarray(x, dtype=np.float32))
    return [
        {"xm": xcs[c], "w1": w1[c], "bias": bias[c]}
        for c in range(NCORES)
    ]


def _gather(results):
    out = np.empty((B, L, C), np.float32)
    for c in range(NCORES):
        oh = results[c]["out"].astype(np.float32)  # [M, NT, NF]
        for t in range(NT):
            base = RPC * c + M * t
            mt = M if t < NT - 1 else M_LAST
            out[:, base : base + mt] = (
                oh[:mt, t].reshape(mt, B, C).transpose(1, 0, 2)
            )
    return out


def kernel(x: np.ndarray, W: np.ndarray, b: np.ndarray) -> np.ndarray:
    nc = _get_nc()
    res = run_bass_kernel_spmd(nc, _make_in_maps(x, W, b), list(range(NCORES)))
    return _gather(res.results)


if __name__ == "__main__":
    rng = np.random.default_rng(0)
    x = rng.standard_normal((B, L, C), dtype=np.float32)
    W = rng.standard_normal((L, PADDED), dtype=np.float32) * 0.02
    b = rng.standard_normal((L,), dtype=np.float32) * 0.02
    print(kernel(x, W, b).shape)
